# revision 29
# baseline (speedup 1.0000x reference)
"""AnomalyAttention (B=4, L=1024, H=8, E=D=64) on 8 TRN2 NeuronCores.

Math: the reference's rfftn/irfftn(s=(H,L,L)) pipeline only keeps frequency
bins f_l in [0,8), f_h in [0,8), f_e in [0,33) (irfftn crops/pads), so the
whole 3D FFT collapses into small DFT matmuls:

  x_ifft[b,m,n,p] = Re{ (1/(8 L L)) sum_{fl,fh,fe} c[fe] X[fl,fh,fe]
                        e^{2pi i fl m/8} e^{2pi i fh n/L} e^{2pi i fe p/L} }
  with X = rfftn(q) * conj(rfftn(k)) on the kept box, c=[1,2,2,...,2].

Each core handles 4 of the 32 (b, h) pairs (b = core//2, h in 4*(core%2)+[0,4)):
fully embarrassingly parallel, no collectives. Per (b,m):
  UW^T (66,1024)  = Xarr^T @ R_m          (Xarr: block-arranged X, R_m: host DFT consts)
  logits x        = UW-cols @ GP          (TensorE, f32r)
  logits x^T      = GP-cols @ UW^T        -> exp -> E^T (bf16)
  V + rowsum Z    = E^T-cols @ [v | 1]    (TensorE, bf16)
  series          = exp(x - ln Z + ln(1/8))   (ScalarE, per-partition bias)
  prior           = exp(-D2/(2 s^2) + ln(c0)) (ScalarE, per-partition scale+bias)
  sigma4          = s broadcast
"""

import math

import numpy as np

B, L, H, E, D = 4, 1024, 8, 64, 64
NF_L, NF_H, NF_E = 8, 8, 33
FHE = NF_H * NF_E            # 264
UWD = 2 * NF_E               # 66
SCALE = 1.0 / math.sqrt(E)
NORM = 1.0 / (L * L)
LN3 = math.log(3.0)
LN_SCALE = math.log(SCALE)
HALF_LN_2PI = 0.5 * math.log(2.0 * math.pi)
P = 128


# ----------------------------------------------------------------- host consts
def _build_wls():
    l = np.arange(L)[:, None]
    f = np.arange(NF_L)[None, :]
    ang = 2.0 * np.pi * l * f / L
    return np.concatenate([np.cos(ang), -np.sin(ang)], axis=1).astype(np.float32)


def _build_m():
    h = np.arange(H)[:, None]
    fh = np.arange(NF_H)[None, :]
    WH = np.exp(-2j * np.pi * h * fh / H)
    e = np.arange(E)[:, None]
    fe = np.arange(NF_E)[None, :]
    WE = np.exp(-2j * np.pi * e * fe / E)
    Mc = np.einsum('hf,eg->hefg', WH, WE).reshape(H * E, FHE)
    return Mc.real.astype(np.float32), np.ascontiguousarray(Mc.imag).astype(np.float32)


def _build_rstk(m_values):
    n = np.arange(L)
    fh = np.arange(NF_H)
    CN = np.cos(2.0 * np.pi * np.outer(fh, n) / L)
    SN = np.sin(2.0 * np.pi * np.outer(fh, n) / L)
    out = []
    for m in m_values:
        fl = np.arange(NF_L)
        cph = np.cos(2.0 * np.pi * fl * m / NF_L)[:, None]
        sph = np.sin(2.0 * np.pi * fl * m / NF_L)[:, None]
        R1 = (cph[:, :, None] * CN[None] - sph[:, :, None] * SN[None]) / 8.0
        R2 = (-sph[:, :, None] * CN[None] - cph[:, :, None] * SN[None]) / 8.0
        R3 = (cph[:, :, None] * SN[None] + sph[:, :, None] * CN[None]) / 8.0
        R1 = R1.reshape(64, L); R2 = R2.reshape(64, L); R3 = R3.reshape(64, L)
        out.append(np.concatenate([R1, R2, R3, R1], axis=0))
    return np.ascontiguousarray(np.stack(out)).astype(np.float32)


def _build_gp():
    p = np.arange(L)
    fe = np.arange(NF_E)
    c = np.ones(NF_E); c[1:] = 2.0
    ang = 2.0 * np.pi * np.outer(fe, p) / L
    gp = np.zeros((P, L), np.float32)
    gp[0:NF_E] = c[:, None] * np.cos(ang) * NORM
    gp[NF_E:UWD] = -c[:, None] * np.sin(ang) * NORM
    return gp


# ------------------------------------------------------------------ bass build
def _emit(nc, tc, mybir, make_identity, h):
    f32 = mybir.dt.float32
    f32r = mybir.dt.float32r
    bf16 = mybir.dt.bfloat16
    Exp = mybir.ActivationFunctionType.Exp
    Log = mybir.ActivationFunctionType.Ln
    MUL = mybir.AluOpType.mult
    ADD = mybir.AluOpType.add
    SUB = mybir.AluOpType.subtract
    import contextlib
    ctx = contextlib.ExitStack()
    with ctx:
        consts = ctx.enter_context(tc.tile_pool(name="consts", bufs=1))
        io = ctx.enter_context(tc.tile_pool(name="io", bufs=1))
        fwdp = ctx.enter_context(tc.tile_pool(name="fwd", bufs=2))
        perm = ctx.enter_context(tc.tile_pool(name="perm", bufs=2))
        etp = ctx.enter_context(tc.tile_pool(name="etp", bufs=2))
        outp = ctx.enter_context(tc.tile_pool(name="outp", bufs=2))
        serp = ctx.enter_context(tc.tile_pool(name="serp", bufs=3))
        smalls = ctx.enter_context(tc.tile_pool(name="smalls", bufs=2))
        bigps = ctx.enter_context(tc.tile_pool(name="bigps", bufs=2, space="PSUM"))
        smallps = ctx.enter_context(tc.tile_pool(name="smallps", bufs=2, space="PSUM"))
        trps = ctx.enter_context(tc.tile_pool(name="trps", bufs=2, space="PSUM"))

        # ---------------- inputs first (q/k unblock the forward DFT ASAP) ----
        q_sb = etp.tile([P, 8, 512], f32r, tag="et")
        nc.scalar.dma_start(out=q_sb, in_=h['q'][:, :].rearrange("(c p) e -> p c e", p=P))
        k_sb = etp.tile([P, 8, 512], f32r, tag="et")
        nc.scalar.dma_start(out=k_sb, in_=h['k'][:, :].rearrange("(c p) e -> p c e", p=P))
        sig_sb = io.tile([P, 8, 4], f32)
        nc.scalar.dma_start(out=sig_sb, in_=h['sig'][:, :].rearrange("(c p) m -> p c m", p=P))
        wls_sb = consts.tile([P, 8, 16], f32r)
        nc.scalar.dma_start(out=wls_sb, in_=h['wls'][:, :].rearrange("(c p) f -> p c f", p=P))
        mre_sb = consts.tile([P, 4, FHE], f32r)
        nc.scalar.dma_start(out=mre_sb, in_=h['mre'][:, :].rearrange("(c p) f -> p c f", p=P))
        mim_sb = consts.tile([P, 4, FHE], f32r)
        nc.scalar.dma_start(out=mim_sb, in_=h['mim'][:, :].rearrange("(c p) f -> p c f", p=P))
        gp_sb = consts.tile([P, L], f32r)
        nc.scalar.dma_start(out=gp_sb, in_=h['gp'][:, :])
        v_sb = io.tile([P, 8, 4, 65], bf16)
        nc.scalar.dma_start(out=v_sb, in_=h['v'][:, :, :].rearrange("(c p) m e -> p c m e", p=P))
        ident = consts.tile([P, P], f32)
        make_identity(nc, ident)

        # ACT table warm-up (natural_log_exp set)
        warm = smalls.tile([P, 1], f32, tag="warm")
        nc.vector.memset(warm, 0.0)
        warm2 = smalls.tile([P, 1], f32, tag="warm2")
        nc.scalar.activation(out=warm2, in_=warm, func=Exp)
        nc.scalar.activation(out=warm, in_=warm2, func=Log)

        # ---------------- D2 = (j - l)^2, built once ----------------
        d0 = io.tile([P, L], f32)
        nc.gpsimd.iota(
            d0, pattern=[[1, L]], base=0,
            channel_multiplier=-1, allow_small_or_imprecise_dtypes=True)
        d2_sb = io.tile([P, 8, L], f32)
        for lc in range(8):
            nc.vector.tensor_scalar(
                out=d2_sb[:, lc, :], in0=d0, scalar1=float(-(lc * P)),
                scalar2=None, op0=ADD)
            nc.gpsimd.tensor_tensor(
                out=d2_sb[:, lc, :], in0=d2_sb[:, lc, :], in1=d2_sb[:, lc, :], op=MUL)

        # ---------------- sigma prep (all 4 m) ----------------
        # s = 3^(sigmoid(5 sig)+1e-5) - 1 ; nega = -1/(2 s^2) ; lbias = -ln(s) - ln(2pi)/2
        t1 = smalls.tile([P, 8, 4], f32, tag="sgp1")
        nc.scalar.activation(out=t1, in_=sig_sb, func=Exp, scale=-5.0)
        nc.vector.tensor_scalar(out=t1, in0=t1, scalar1=1.0, scalar2=None, op0=ADD)
        sgm = smalls.tile([P, 8, 4], f32, tag="sgp2")
        nc.vector.reciprocal(out=sgm, in_=t1)
        s_all = io.tile([P, 8, 4], f32)
        b3 = smalls.tile([P, 1], f32, tag="b3")
        nc.vector.memset(b3, 1e-5 * LN3)
        nc.scalar.activation(out=s_all, in_=sgm, func=Exp, scale=LN3, bias=b3)
        nc.vector.tensor_scalar(out=s_all, in0=s_all, scalar1=-1.0, scalar2=None, op0=ADD)
        s2 = smalls.tile([P, 8, 4], f32, tag="sgp3")
        nc.vector.tensor_tensor(out=s2, in0=s_all, in1=s_all, op=MUL)
        rs2 = smalls.tile([P, 8, 4], f32, tag="sgp4")
        nc.vector.reciprocal(out=rs2, in_=s2)
        nega_all = io.tile([P, 8, 4], f32)
        nc.vector.tensor_scalar(out=nega_all, in0=rs2, scalar1=-0.5, scalar2=None, op0=MUL)
        lg = smalls.tile([P, 8, 4], f32, tag="sgp5")
        nc.scalar.activation(out=lg, in_=s_all, func=Log)
        lbias_all = io.tile([P, 8, 4], f32)
        nc.vector.tensor_scalar(
            out=lbias_all, in0=lg, scalar1=-1.0, scalar2=-HALF_LN_2PI, op0=MUL, op1=ADD)

        # ---------------- forward DFT smalls ----------------
        at = {}
        for name, src in (("q", q_sb), ("k", k_sb)):
            at_sb = fwdp.tile([P, 4, 16], f32r, tag="at")
            for hs in range(4):
                ps = smallps.tile([P, 16], f32, tag="sps")
                for lcc in range(8):
                    nc.tensor.matmul(
                        ps, lhsT=src[:, lcc, hs * P:(hs + 1) * P],
                        rhs=wls_sb[:, lcc, :], start=(lcc == 0), stop=(lcc == 7))
                nc.vector.tensor_copy(out=at_sb[:, hs, :], in_=ps)
            at[name] = at_sb

        bt = {}
        FCS = ((0, 128), (1, 128), (2, 8))
        for name in ("q", "k"):
            bsb = fwdp.tile([P, 3, 16], f32, tag="bt")
            for fc, sz in FCS:
                p1 = smallps.tile([P, 16], f32, tag="sps")
                p2 = smallps.tile([P, 16], f32, tag="sps")
                for hc in range(4):
                    nc.tensor.matmul(
                        p1[:sz], lhsT=mre_sb[:, hc, fc * P:fc * P + sz],
                        rhs=at[name][:, hc, :], start=(hc == 0), stop=(hc == 3))
                for hc in range(4):
                    nc.tensor.matmul(
                        p2[:sz], lhsT=mim_sb[:, hc, fc * P:fc * P + sz],
                        rhs=at[name][:, hc, :], start=(hc == 0), stop=(hc == 3))
                p1s = smalls.tile([P, 16], f32, tag="p1s")
                nc.vector.tensor_copy(out=p1s[:sz], in_=p1[:sz])
                nc.vector.tensor_tensor(
                    out=bsb[:sz, fc, 0:8], in0=p1s[:sz, 0:8], in1=p2[:sz, 8:16], op=SUB)
                nc.vector.tensor_tensor(
                    out=bsb[:sz, fc, 8:16], in0=p1s[:sz, 8:16], in1=p2[:sz, 0:8], op=ADD)
            bt[name] = bsb

        # X = Bq * conj(Bk)
        xc = fwdp.tile([P, 3, 16], f32, tag="xc")
        for fc, sz in FCS:
            rq = bt["q"][:sz, fc, 0:8]; iq = bt["q"][:sz, fc, 8:16]
            rk = bt["k"][:sz, fc, 0:8]; ik = bt["k"][:sz, fc, 8:16]
            ta = smalls.tile([P, 8], f32, tag="xta")
            tb = smalls.tile([P, 8], f32, tag="xtb")
            nc.vector.tensor_tensor(out=ta[:sz], in0=rq, in1=rk, op=MUL)
            nc.vector.tensor_tensor(out=tb[:sz], in0=iq, in1=ik, op=MUL)
            nc.vector.tensor_tensor(out=xc[:sz, fc, 0:8], in0=ta[:sz], in1=tb[:sz], op=ADD)
            nc.vector.tensor_tensor(out=ta[:sz], in0=iq, in1=rk, op=MUL)
            nc.vector.tensor_tensor(out=tb[:sz], in0=rq, in1=ik, op=MUL)
            nc.vector.tensor_tensor(out=xc[:sz, fc, 8:16], in0=ta[:sz], in1=tb[:sz], op=SUB)

        # transpose X -> (16, 264) f32r (DVE copy rounds)
        xt_sb = fwdp.tile([16, FHE], f32r, tag="xtT")
        for fc, sz in FCS:
            pt = trps.tile([16, P], f32, tag="spsr")
            nc.tensor.transpose(pt, xc[:, fc, :], ident)
            nc.vector.tensor_copy(out=xt_sb[:, fc * P:fc * P + sz], in_=pt[:, :sz])

        # Build the two block-structured Xarr chunks in a DRAM scratch: all the
        # (fl,fh)->partition restructuring happens on linear DRAM APs, then a
        # plain 2D DMA loads each chunk back.  xarrd layout: (2, 128, 66).
        zeros33 = fwdp.tile([P, NF_E], f32r, tag="z33")
        nc.vector.tensor_scalar(
            out=zeros33, in0=d0[:, 0:NF_E], scalar1=0.0, scalar2=None, op0=MUL)
        nc.sync.dma_start(out=h['xarrd'][0, :, NF_E:UWD], in_=zeros33)
        nc.sync.dma_start(out=h['xarrd'][1, :, 0:NF_E], in_=zeros33)
        for cc, cols in ((0, (0, NF_E)), (1, (NF_E, UWD))):
            for ri, xrows in ((0, (0, 8)), (1, (8, 16))):
                nc.sync.dma_start(
                    out=h['xarrd'][cc, ri * 64:(ri + 1) * 64,
                                   cols[0]:cols[1]].rearrange(
                        "(fl fh) fe -> fl fh fe", fh=8),
                    in_=xt_sb[xrows[0]:xrows[1], :].rearrange(
                        "fl (fh fe) -> fl fh fe", fe=NF_E))
        xarr = []
        for cc in range(2):
            xa = fwdp.tile([P, UWD], f32r, tag=f"xarr{cc}")
            nc.scalar.dma_start(out=xa, in_=h['xarrd'][cc, :, :])
            xarr.append(xa)

        # two rotating UW^T buffers with junk rows pre-zeroed once
        uwsbs = []
        for i in range(2):
            u = io.tile([P, L], f32r, tag=f"uwsb{i}")
            nc.vector.tensor_scalar(
                out=u[64:P, :], in0=d0[64:P, :], scalar1=0.0, scalar2=None, op0=MUL)
            uwsbs.append(u)

        # ---------------- per-m heavy pipeline ----------------
        for mi in range(4):
            rm_sb = perm.tile([P, 2, L], f32r, tag="rm")
            nc.scalar.dma_start(
                out=rm_sb, in_=h['rstk'][mi, :, :].rearrange("(c p) n -> p c n", p=P))

            # UW^T (66, 1024)
            uwsb = uwsbs[mi % 2]
            for hh in range(2):
                uw_ps = smallps.tile([UWD, 512], f32, tag="sps")
                for cc in range(2):
                    nc.tensor.matmul(
                        uw_ps, lhsT=xarr[cc],
                        rhs=rm_sb[:, cc, hh * 512:(hh + 1) * 512],
                        start=(cc == 0), stop=(cc == 1))
                nc.vector.tensor_copy(out=uwsb[0:UWD, hh * 512:(hh + 1) * 512], in_=uw_ps)

            # prior = exp(nega*D2 + lbias)
            for pair in range(4):
                prsb = outp.tile([P, 2, L], f32, tag="pri")
                for sub in range(2):
                    lc = pair * 2 + sub
                    nc.scalar.activation(
                        out=prsb[:, sub, :], in_=d2_sb[:, lc, :], func=Exp,
                        scale=nega_all[:, lc, mi:mi + 1], bias=lbias_all[:, lc, mi:mi + 1])
                nc.sync.dma_start(
                    out=h['pri'][mi, pair * 256:(pair + 1) * 256, :].rearrange(
                        "(c p) s -> p c s", p=P),
                    in_=prsb)

            # x^T -> E^T (bf16)
            et_sb = etp.tile([P, 8, L], bf16, tag="et")
            for pc in range(8):
                xt_ps = bigps.tile([P, L], f32, tag="bigps")
                for hh in range(2):
                    nc.tensor.matmul(
                        xt_ps[:, hh * 512:(hh + 1) * 512],
                        lhsT=gp_sb[:, pc * P:(pc + 1) * P],
                        rhs=uwsb[:, hh * 512:(hh + 1) * 512], start=True, stop=True)
                nc.scalar.activation(out=et_sb[:, pc, :], in_=xt_ps, func=Exp)

            # V^T = [v|1]^T-stationary @ E^T-moving, contraction over s, then
            # transpose 128-col blocks back so V lands with l on partitions.
            vt_sb = perm.tile([P, L], f32, tag="vt")
            for hh in range(2):
                vt_ps = smallps.tile([65, 512], f32, tag="sps")
                for sc in range(8):
                    nc.tensor.matmul(
                        vt_ps, lhsT=v_sb[:, sc, mi, :],
                        rhs=et_sb[:, sc, hh * 512:(hh + 1) * 512],
                        start=(sc == 0), stop=(sc == 7))
                nc.vector.tensor_copy(out=vt_sb[0:65, hh * 512:(hh + 1) * 512], in_=vt_ps)
            vsb = outp.tile([P, 8, 64], f32, tag="vsb")
            ztile = smalls.tile([P, 8], f32, tag="zt")
            rz = smalls.tile([P, 8], f32, tag="rz")
            for lc in range(8):
                v_ps = trps.tile([P, 65], f32, tag="spsr")
                nc.tensor.transpose(
                    v_ps, vt_sb[0:65, lc * P:(lc + 1) * P], ident[0:65, 0:65])
                nc.vector.tensor_copy(out=ztile[:, lc:lc + 1], in_=v_ps[:, 64:65])
                nc.vector.reciprocal(out=rz[:, lc:lc + 1], in_=v_ps[:, 64:65])
                nc.vector.tensor_scalar(
                    out=vsb[:, lc, :], in0=v_ps[:, 0:64],
                    scalar1=rz[:, lc:lc + 1], scalar2=SCALE, op0=MUL, op1=MUL)
            lnz = smalls.tile([P, 8], f32, tag="lnz")
            nc.scalar.activation(out=lnz, in_=ztile, func=Log)
            serbias = smalls.tile([P, 8], f32, tag="serbias")
            nc.vector.tensor_scalar(
                out=serbias, in0=lnz, scalar1=-1.0, scalar2=LN_SCALE, op0=MUL, op1=ADD)
            nc.sync.dma_start(
                out=h['vout'][:, mi, :].rearrange("(c p) d -> p c d", p=P), in_=vsb)

            # x -> series = exp(x + serbias)
            for pair in range(4):
                sersb = serp.tile([P, 2, L], f32, tag="ser")
                for sub in range(2):
                    lc = pair * 2 + sub
                    x_ps = bigps.tile([P, L], f32, tag="bigps")
                    for hh in range(2):
                        nc.tensor.matmul(
                            x_ps[:, hh * 512:(hh + 1) * 512],
                            lhsT=uwsb[:, lc * P:(lc + 1) * P],
                            rhs=gp_sb[:, hh * 512:(hh + 1) * 512], start=True, stop=True)
                    nc.scalar.activation(
                        out=sersb[:, sub, :], in_=x_ps, func=Exp,
                        bias=serbias[:, lc:lc + 1])
                nc.sync.dma_start(
                    out=h['ser'][mi, pair * 256:(pair + 1) * 256, :].rearrange(
                        "(c p) s -> p c s", p=P),
                    in_=sersb)

            # sigma4 = s broadcast
            for pair in range(4):
                sgsb = outp.tile([P, 2, L], f32, tag="sg4")
                for sub in range(2):
                    lc = pair * 2 + sub
                    nc.vector.tensor_copy(
                        out=sgsb[:, sub, :],
                        in_=s_all[:, lc, mi:mi + 1].to_broadcast((P, L)))
                nc.sync.dma_start(
                    out=h['sg4'][mi, pair * 256:(pair + 1) * 256, :].rearrange(
                        "(c p) s -> p c s", p=P),
                    in_=sgsb)


_PROGRAM = None


def _build_program():
    global _PROGRAM
    if _PROGRAM is not None:
        return _PROGRAM
    import concourse.mybir as mybir
    import concourse.tile as tile
    from concourse import bacc
    from concourse.masks import make_identity

    f32 = mybir.dt.float32
    f32r = mybir.dt.float32r
    bf16 = mybir.dt.bfloat16

    nc = bacc.Bacc(
        "TRN2", target_bir_lowering=False, debug=False, num_devices=8)
    h = {}
    h['q'] = nc.declare_dram_parameter("q", [L, H * E], f32r, isOutput=False)
    h['k'] = nc.declare_dram_parameter("k", [L, H * E], f32r, isOutput=False)
    h['v'] = nc.declare_dram_parameter("v", [L, 4, 65], bf16, isOutput=False)
    h['sig'] = nc.declare_dram_parameter("sig", [L, 4], f32, isOutput=False)
    h['wls'] = nc.declare_dram_parameter("wls", [L, 16], f32r, isOutput=False)
    h['mre'] = nc.declare_dram_parameter("mre", [H * E, FHE], f32r, isOutput=False)
    h['mim'] = nc.declare_dram_parameter("mim", [H * E, FHE], f32r, isOutput=False)
    h['rstk'] = nc.declare_dram_parameter("rstk", [4, 256, L], f32r, isOutput=False)
    h['gp'] = nc.declare_dram_parameter("gp", [P, L], f32r, isOutput=False)
    h['xarrd'] = nc.dram_tensor("xarrd", [2, P, UWD], f32r)
    h['vout'] = nc.declare_dram_parameter("vout", [L, 4, D], f32, isOutput=True)
    h['ser'] = nc.declare_dram_parameter("ser", [4, L, L], f32, isOutput=True)
    h['pri'] = nc.declare_dram_parameter("pri", [4, L, L], f32, isOutput=True)
    h['sg4'] = nc.declare_dram_parameter("sg4", [4, L, L], f32, isOutput=True)

    with tile.TileContext(nc) as tc:
        _emit(nc, tc, mybir, make_identity, h)
    nc.compile()
    _PROGRAM = nc
    return nc


def kernel(queries, keys, values, sigma):
    import ml_dtypes
    from concourse.bass_utils import run_bass_kernel_spmd

    queries = np.asarray(queries, np.float32)
    keys = np.asarray(keys, np.float32)
    values = np.asarray(values, np.float32)
    sigma = np.asarray(sigma, np.float32)

    nc = _build_program()

    wls = _build_wls()
    mre, mim = _build_m()
    gp = _build_gp()
    rstk_half = {0: _build_rstk([0, 1, 2, 3]), 1: _build_rstk([4, 5, 6, 7])}

    in_maps = []
    for c in range(8):
        b, mh = c // 2, c % 2
        m0 = 4 * mh
        vsh = np.empty((L, 4, 65), np.float32)
        vsh[:, :, :64] = values[b][:, m0:m0 + 4, :]
        vsh[:, :, 64] = 1.0
        in_maps.append(dict(
            q=np.ascontiguousarray(queries[b].reshape(L, H * E)),
            k=np.ascontiguousarray(keys[b].reshape(L, H * E)),
            v=vsh.astype(ml_dtypes.bfloat16),
            sig=np.ascontiguousarray(sigma[b][:, m0:m0 + 4]),
            wls=wls, mre=mre, mim=mim, rstk=rstk_half[mh], gp=gp,
        ))

    res = run_bass_kernel_spmd(nc, in_maps, core_ids=list(range(8))).results

    V = np.empty((B, L, H, D), np.float32)
    series = np.empty((B, H, L, L), np.float32)
    prior = np.empty((B, H, L, L), np.float32)
    sigma4 = np.empty((B, H, L, L), np.float32)
    for c in range(8):
        b, m0 = c // 2, 4 * (c % 2)
        V[b][:, m0:m0 + 4, :] = res[c]['vout']
        series[b, m0:m0 + 4] = res[c]['ser']
        prior[b, m0:m0 + 4] = res[c]['pri']
        sigma4[b, m0:m0 + 4] = res[c]['sg4']
    return V, series, prior, sigma4


# revision 32
# speedup vs baseline: 1.1532x; 1.1532x over previous
"""AnomalyAttention (B=4, L=1024, H=8, E=D=64) on 8 TRN2 NeuronCores.

Math: the reference's rfftn/irfftn(s=(H,L,L)) pipeline only keeps frequency
bins f_l in [0,8), f_h in [0,8), f_e in [0,33) (irfftn crops/pads), so the
whole 3D FFT collapses into small DFT matmuls:

  x_ifft[b,m,n,p] = Re{ (1/(8 L L)) sum_{fl,fh,fe} c[fe] X[fl,fh,fe]
                        e^{2pi i fl m/8} e^{2pi i fh n/L} e^{2pi i fe p/L} }
  with X = rfftn(q) * conj(rfftn(k)) on the kept box, c=[1,2,2,...,2].

Each core handles 4 of the 32 (b, h) pairs (b = core//2, h in 4*(core%2)+[0,4)):
fully embarrassingly parallel, no collectives. Per (b,m):
  UW^T (66,1024)  = Xarr^T @ R_m          (Xarr: block-arranged X, R_m: host DFT consts)
  logits x        = UW-cols @ GP          (TensorE, f32r)
  logits x^T      = GP-cols @ UW^T        -> exp -> E^T (bf16)
  V + rowsum Z    = E^T-cols @ [v | 1]    (TensorE, bf16)
  series          = exp(x - ln Z + ln(1/8))   (ScalarE, per-partition bias)
  prior           = exp(-D2/(2 s^2) + ln(c0)) (ScalarE, per-partition scale+bias)
  sigma4          = s broadcast
"""

import math

import numpy as np

B, L, H, E, D = 4, 1024, 8, 64, 64
NF_L, NF_H, NF_E = 8, 8, 33
FHE = NF_H * NF_E            # 264
UWD = 2 * NF_E               # 66
SCALE = 1.0 / math.sqrt(E)
NORM = 1.0 / (L * L)
LN3 = math.log(3.0)
LN_SCALE = math.log(SCALE)
HALF_LN_2PI = 0.5 * math.log(2.0 * math.pi)
P = 128


# ----------------------------------------------------------------- host consts
def _build_wls():
    l = np.arange(L)[:, None]
    f = np.arange(NF_L)[None, :]
    ang = 2.0 * np.pi * l * f / L
    return np.concatenate([np.cos(ang), -np.sin(ang)], axis=1).astype(np.float32)


def _build_m():
    h = np.arange(H)[:, None]
    fh = np.arange(NF_H)[None, :]
    WH = np.exp(-2j * np.pi * h * fh / H)
    e = np.arange(E)[:, None]
    fe = np.arange(NF_E)[None, :]
    WE = np.exp(-2j * np.pi * e * fe / E)
    Mc = np.einsum('hf,eg->hefg', WH, WE).reshape(H * E, FHE)
    return Mc.real.astype(np.float32), np.ascontiguousarray(Mc.imag).astype(np.float32)


def _build_rstk(m_values):
    n = np.arange(L)
    fh = np.arange(NF_H)
    CN = np.cos(2.0 * np.pi * np.outer(fh, n) / L)
    SN = np.sin(2.0 * np.pi * np.outer(fh, n) / L)
    out = []
    for m in m_values:
        fl = np.arange(NF_L)
        cph = np.cos(2.0 * np.pi * fl * m / NF_L)[:, None]
        sph = np.sin(2.0 * np.pi * fl * m / NF_L)[:, None]
        R1 = (cph[:, :, None] * CN[None] - sph[:, :, None] * SN[None]) / 8.0
        R2 = (-sph[:, :, None] * CN[None] - cph[:, :, None] * SN[None]) / 8.0
        R3 = (cph[:, :, None] * SN[None] + sph[:, :, None] * CN[None]) / 8.0
        R1 = R1.reshape(64, L); R2 = R2.reshape(64, L); R3 = R3.reshape(64, L)
        out.append(np.concatenate([R1, R2, R3, R1], axis=0))
    return np.ascontiguousarray(np.stack(out)).astype(np.float32)


def _build_gp():
    p = np.arange(L)
    fe = np.arange(NF_E)
    c = np.ones(NF_E); c[1:] = 2.0
    ang = 2.0 * np.pi * np.outer(fe, p) / L
    gp = np.zeros((P, L), np.float32)
    gp[0:NF_E] = c[:, None] * np.cos(ang) * NORM
    gp[NF_E:UWD] = -c[:, None] * np.sin(ang) * NORM
    return gp


# ------------------------------------------------------------------ bass build
def _emit(nc, tc, mybir, make_identity, h):
    f32 = mybir.dt.float32
    f32r = mybir.dt.float32r
    bf16 = mybir.dt.bfloat16
    Exp = mybir.ActivationFunctionType.Exp
    Log = mybir.ActivationFunctionType.Ln
    MUL = mybir.AluOpType.mult
    ADD = mybir.AluOpType.add
    SUB = mybir.AluOpType.subtract
    import contextlib
    ctx = contextlib.ExitStack()
    with ctx:
        consts = ctx.enter_context(tc.tile_pool(name="consts", bufs=1))
        io = ctx.enter_context(tc.tile_pool(name="io", bufs=1))
        fwdp = ctx.enter_context(tc.tile_pool(name="fwd", bufs=2))
        perm = ctx.enter_context(tc.tile_pool(name="perm", bufs=2))
        etp = ctx.enter_context(tc.tile_pool(name="etp", bufs=2))
        outp = ctx.enter_context(tc.tile_pool(name="outp", bufs=2))
        serp = ctx.enter_context(tc.tile_pool(name="serp", bufs=3))
        smalls = ctx.enter_context(tc.tile_pool(name="smalls", bufs=2))
        bigps = ctx.enter_context(tc.tile_pool(name="bigps", bufs=2, space="PSUM"))
        smallps = ctx.enter_context(tc.tile_pool(name="smallps", bufs=2, space="PSUM"))
        trps = ctx.enter_context(tc.tile_pool(name="trps", bufs=2, space="PSUM"))

        # ---------------- inputs first (q/k unblock the forward DFT ASAP) ----
        q_sb = etp.tile([P, 8, 512], f32r, tag="et")
        nc.scalar.dma_start(out=q_sb, in_=h['q'][:, :].rearrange("(c p) e -> p c e", p=P))
        k_sb = etp.tile([P, 8, 512], f32r, tag="et")
        nc.scalar.dma_start(out=k_sb, in_=h['k'][:, :].rearrange("(c p) e -> p c e", p=P))
        sig_sb = io.tile([P, 8, 4], f32)
        nc.scalar.dma_start(out=sig_sb, in_=h['sig'][:, :].rearrange("(c p) m -> p c m", p=P))
        wls_sb = consts.tile([P, 8, 16], f32r)
        nc.scalar.dma_start(out=wls_sb, in_=h['wls'][:, :].rearrange("(c p) f -> p c f", p=P))
        mre_sb = consts.tile([P, 4, FHE], f32r)
        nc.scalar.dma_start(out=mre_sb, in_=h['mre'][:, :].rearrange("(c p) f -> p c f", p=P))
        mim_sb = consts.tile([P, 4, FHE], f32r)
        nc.scalar.dma_start(out=mim_sb, in_=h['mim'][:, :].rearrange("(c p) f -> p c f", p=P))
        gp_sb = consts.tile([P, L], f32r)
        nc.scalar.dma_start(out=gp_sb, in_=h['gp'][:, :])
        v_sb = io.tile([P, 8, 4, 65], bf16)
        nc.scalar.dma_start(out=v_sb, in_=h['v'][:, :, :].rearrange("(c p) m e -> p c m e", p=P))
        ident = consts.tile([P, P], f32)
        make_identity(nc, ident)

        # ACT table warm-up (natural_log_exp set)
        warm = smalls.tile([P, 1], f32, tag="warm")
        nc.vector.memset(warm, 0.0)
        warm2 = smalls.tile([P, 1], f32, tag="warm2")
        nc.scalar.activation(out=warm2, in_=warm, func=Exp)
        nc.scalar.activation(out=warm, in_=warm2, func=Log)

        # ---------------- D2 = (j - l)^2, built once ----------------
        d0 = io.tile([P, L], f32)
        nc.gpsimd.iota(
            d0, pattern=[[1, L]], base=0,
            channel_multiplier=-1, allow_small_or_imprecise_dtypes=True)
        d2_sb = io.tile([P, 8, L], f32)
        for lc in range(8):
            nc.vector.tensor_scalar(
                out=d2_sb[:, lc, :], in0=d0, scalar1=float(-(lc * P)),
                scalar2=None, op0=ADD)
            nc.gpsimd.tensor_tensor(
                out=d2_sb[:, lc, :], in0=d2_sb[:, lc, :], in1=d2_sb[:, lc, :], op=MUL)

        # ---------------- sigma prep (all 4 m) ----------------
        # s = 3^(sigmoid(5 sig)+1e-5) - 1 ; nega = -1/(2 s^2) ; lbias = -ln(s) - ln(2pi)/2
        t1 = smalls.tile([P, 8, 4], f32, tag="sgp1")
        nc.scalar.activation(out=t1, in_=sig_sb, func=Exp, scale=-5.0)
        nc.vector.tensor_scalar(out=t1, in0=t1, scalar1=1.0, scalar2=None, op0=ADD)
        sgm = smalls.tile([P, 8, 4], f32, tag="sgp2")
        nc.vector.reciprocal(out=sgm, in_=t1)
        s_all = io.tile([P, 8, 4], f32)
        b3 = smalls.tile([P, 1], f32, tag="b3")
        nc.vector.memset(b3, 1e-5 * LN3)
        nc.scalar.activation(out=s_all, in_=sgm, func=Exp, scale=LN3, bias=b3)
        nc.vector.tensor_scalar(out=s_all, in0=s_all, scalar1=-1.0, scalar2=None, op0=ADD)
        s2 = smalls.tile([P, 8, 4], f32, tag="sgp3")
        nc.vector.tensor_tensor(out=s2, in0=s_all, in1=s_all, op=MUL)
        rs2 = smalls.tile([P, 8, 4], f32, tag="sgp4")
        nc.vector.reciprocal(out=rs2, in_=s2)
        nega_all = io.tile([P, 8, 4], f32)
        nc.vector.tensor_scalar(out=nega_all, in0=rs2, scalar1=-0.5, scalar2=None, op0=MUL)
        lg = smalls.tile([P, 8, 4], f32, tag="sgp5")
        nc.scalar.activation(out=lg, in_=s_all, func=Log)
        lbias_all = io.tile([P, 8, 4], f32)
        nc.vector.tensor_scalar(
            out=lbias_all, in0=lg, scalar1=-1.0, scalar2=-HALF_LN_2PI, op0=MUL, op1=ADD)

        # ---------------- forward DFT smalls ----------------
        at = {}
        for name, src in (("q", q_sb), ("k", k_sb)):
            at_sb = fwdp.tile([P, 4, 16], f32r, tag="at")
            for hs in range(4):
                ps = smallps.tile([P, 16], f32, tag="sps")
                for lcc in range(8):
                    nc.tensor.matmul(
                        ps, lhsT=src[:, lcc, hs * P:(hs + 1) * P],
                        rhs=wls_sb[:, lcc, :], start=(lcc == 0), stop=(lcc == 7))
                nc.vector.tensor_copy(out=at_sb[:, hs, :], in_=ps)
            at[name] = at_sb

        bt = {}
        FCS = ((0, 128), (1, 128), (2, 8))
        for name in ("q", "k"):
            bsb = fwdp.tile([P, 3, 16], f32, tag="bt")
            for fc, sz in FCS:
                p1 = smallps.tile([P, 16], f32, tag="sps")
                p2 = smallps.tile([P, 16], f32, tag="sps")
                for hc in range(4):
                    nc.tensor.matmul(
                        p1[:sz], lhsT=mre_sb[:, hc, fc * P:fc * P + sz],
                        rhs=at[name][:, hc, :], start=(hc == 0), stop=(hc == 3))
                for hc in range(4):
                    nc.tensor.matmul(
                        p2[:sz], lhsT=mim_sb[:, hc, fc * P:fc * P + sz],
                        rhs=at[name][:, hc, :], start=(hc == 0), stop=(hc == 3))
                p1s = smalls.tile([P, 16], f32, tag="p1s")
                nc.vector.tensor_copy(out=p1s[:sz], in_=p1[:sz])
                nc.vector.tensor_tensor(
                    out=bsb[:sz, fc, 0:8], in0=p1s[:sz, 0:8], in1=p2[:sz, 8:16], op=SUB)
                nc.vector.tensor_tensor(
                    out=bsb[:sz, fc, 8:16], in0=p1s[:sz, 8:16], in1=p2[:sz, 0:8], op=ADD)
            bt[name] = bsb

        # X = Bq * conj(Bk)
        xc = fwdp.tile([P, 3, 16], f32, tag="xc")
        for fc, sz in FCS:
            rq = bt["q"][:sz, fc, 0:8]; iq = bt["q"][:sz, fc, 8:16]
            rk = bt["k"][:sz, fc, 0:8]; ik = bt["k"][:sz, fc, 8:16]
            ta = smalls.tile([P, 8], f32, tag="xta")
            tb = smalls.tile([P, 8], f32, tag="xtb")
            nc.vector.tensor_tensor(out=ta[:sz], in0=rq, in1=rk, op=MUL)
            nc.vector.tensor_tensor(out=tb[:sz], in0=iq, in1=ik, op=MUL)
            nc.vector.tensor_tensor(out=xc[:sz, fc, 0:8], in0=ta[:sz], in1=tb[:sz], op=ADD)
            nc.vector.tensor_tensor(out=ta[:sz], in0=iq, in1=rk, op=MUL)
            nc.vector.tensor_tensor(out=tb[:sz], in0=rq, in1=ik, op=MUL)
            nc.vector.tensor_tensor(out=xc[:sz, fc, 8:16], in0=ta[:sz], in1=tb[:sz], op=SUB)

        # transpose X -> (16, 264) f32r (DVE copy rounds)
        xt_sb = fwdp.tile([16, FHE], f32r, tag="xtT")
        for fc, sz in FCS:
            pt = trps.tile([16, P], f32, tag="spsr")
            nc.tensor.transpose(pt, xc[:, fc, :], ident)
            nc.vector.tensor_copy(out=xt_sb[:, fc * P:fc * P + sz], in_=pt[:, :sz])

        # Build the two block-structured Xarr chunks in a DRAM scratch: all the
        # (fl,fh)->partition restructuring happens on linear DRAM APs, then a
        # plain 2D DMA loads each chunk back.  xarrd layout: (2, 128, 66).
        zeros33 = fwdp.tile([P, NF_E], f32r, tag="z33")
        nc.vector.tensor_scalar(
            out=zeros33, in0=d0[:, 0:NF_E], scalar1=0.0, scalar2=None, op0=MUL)
        nc.sync.dma_start(out=h['xarrd'][0, :, NF_E:UWD], in_=zeros33)
        nc.sync.dma_start(out=h['xarrd'][1, :, 0:NF_E], in_=zeros33)
        for cc, cols in ((0, (0, NF_E)), (1, (NF_E, UWD))):
            for ri, xrows in ((0, (0, 8)), (1, (8, 16))):
                nc.sync.dma_start(
                    out=h['xarrd'][cc, ri * 64:(ri + 1) * 64,
                                   cols[0]:cols[1]].rearrange(
                        "(fl fh) fe -> fl fh fe", fh=8),
                    in_=xt_sb[xrows[0]:xrows[1], :].rearrange(
                        "fl (fh fe) -> fl fh fe", fe=NF_E))
        xarr = []
        for cc in range(2):
            xa = fwdp.tile([P, UWD], f32r, tag=f"xarr{cc}")
            nc.scalar.dma_start(out=xa, in_=h['xarrd'][cc, :, :])
            xarr.append(xa)

        # two rotating UW^T buffers with junk rows pre-zeroed once
        uwsbs = []
        for i in range(2):
            u = io.tile([P, L], f32r, tag=f"uwsb{i}")
            nc.vector.tensor_scalar(
                out=u[64:P, :], in0=d0[64:P, :], scalar1=0.0, scalar2=None, op0=MUL)
            uwsbs.append(u)

        # ---------------- per-m heavy pipeline ----------------
        for mi in range(4):
            rm_sb = perm.tile([P, 2, L], f32r, tag="rm")
            nc.scalar.dma_start(
                out=rm_sb, in_=h['rstk'][mi, :, :].rearrange("(c p) n -> p c n", p=P))

            # UW^T (66, 1024)
            uwsb = uwsbs[mi % 2]
            for hh in range(2):
                uw_ps = smallps.tile([UWD, 512], f32, tag="sps")
                for cc in range(2):
                    nc.tensor.matmul(
                        uw_ps, lhsT=xarr[cc],
                        rhs=rm_sb[:, cc, hh * 512:(hh + 1) * 512],
                        start=(cc == 0), stop=(cc == 1))
                nc.vector.tensor_copy(out=uwsb[0:UWD, hh * 512:(hh + 1) * 512], in_=uw_ps)

            # prior = exp(nega*D2 + lbias)
            for pair in range(4):
                prsb = outp.tile([P, 2, L], f32, tag="pri")
                for sub in range(2):
                    lc = pair * 2 + sub
                    nc.scalar.activation(
                        out=prsb[:, sub, :], in_=d2_sb[:, lc, :], func=Exp,
                        scale=nega_all[:, lc, mi:mi + 1], bias=lbias_all[:, lc, mi:mi + 1])
                nc.sync.dma_start(
                    out=h['pri'][mi, pair * 256:(pair + 1) * 256, :].rearrange(
                        "(c p) s -> p c s", p=P),
                    in_=prsb)

            # x^T -> E^T (bf16)
            et_sb = etp.tile([P, 8, L], bf16, tag="et")
            for pc in range(8):
                xt_ps = bigps.tile([P, L], f32, tag="bigps")
                for hh in range(2):
                    nc.tensor.matmul(
                        xt_ps[:, hh * 512:(hh + 1) * 512],
                        lhsT=gp_sb[:, pc * P:(pc + 1) * P],
                        rhs=uwsb[:, hh * 512:(hh + 1) * 512], start=True, stop=True)
                nc.scalar.activation(out=et_sb[:, pc, :], in_=xt_ps, func=Exp)

            # V^T = [v|1]^T-stationary @ E^T-moving, contraction over s, then
            # transpose 128-col blocks back so V lands with l on partitions.
            vt_sb = perm.tile([P, L], f32, tag="vt")
            for hh in range(2):
                vt_ps = smallps.tile([65, 512], f32, tag="sps")
                for sc in range(8):
                    nc.tensor.matmul(
                        vt_ps, lhsT=v_sb[:, sc, mi, :],
                        rhs=et_sb[:, sc, hh * 512:(hh + 1) * 512],
                        start=(sc == 0), stop=(sc == 7))
                nc.vector.tensor_copy(out=vt_sb[0:65, hh * 512:(hh + 1) * 512], in_=vt_ps)
            vsb = outp.tile([P, 8, 64], f32, tag="vsb")
            ztile = smalls.tile([P, 8], f32, tag="zt")
            rz = smalls.tile([P, 8], f32, tag="rz")
            for lc in range(8):
                v_ps = trps.tile([P, 65], f32, tag="spsr")
                nc.tensor.transpose(
                    v_ps, vt_sb[0:65, lc * P:(lc + 1) * P], ident[0:65, 0:65])
                nc.vector.tensor_copy(out=ztile[:, lc:lc + 1], in_=v_ps[:, 64:65])
                nc.vector.reciprocal(out=rz[:, lc:lc + 1], in_=v_ps[:, 64:65])
                nc.vector.tensor_scalar(
                    out=vsb[:, lc, :], in0=v_ps[:, 0:64],
                    scalar1=rz[:, lc:lc + 1], scalar2=SCALE, op0=MUL, op1=MUL)
            nc.sync.dma_start(
                out=h['vout'][:, mi, :].rearrange("(c p) d -> p c d", p=P), in_=vsb)

            # series^T = E^T * (scale/Z_l): replicate the 1/Z row across all
            # partitions via a rank-1 PE outer product, then scale E^T tiles.
            # The host transposes ser back during unshard.
            rzrow = smalls.tile([1, L], f32r, tag="rzrow")
            nc.vector.reciprocal(out=rzrow.bitcast(f32), in_=vt_sb[64:65, :])
            nc.vector.tensor_scalar(
                out=rzrow.bitcast(f32), in0=rzrow.bitcast(f32),
                scalar1=SCALE, scalar2=None, op0=MUL)
            rzfull = smalls.tile([P, L], f32, tag="rzfull")
            nc.scalar.dma_start(out=h['rzd'][:, :], in_=rzrow.bitcast(f32))
            nc.scalar.dma_start(
                out=rzfull, in_=h['rzd'][:, :].to_broadcast((P, L)))
            for pair in range(4):
                sersb = serp.tile([P, 2, L], f32, tag="ser")
                for sub in range(2):
                    sc = pair * 2 + sub
                    nc.vector.tensor_tensor(
                        out=sersb[:, sub, :], in0=et_sb[:, sc, :],
                        in1=rzfull, op=MUL)
                nc.sync.dma_start(
                    out=h['ser'][mi, pair * 256:(pair + 1) * 256, :].rearrange(
                        "(c p) s -> p c s", p=P),
                    in_=sersb)

            # sigma4 = s broadcast
            for pair in range(4):
                sgsb = outp.tile([P, 2, L], f32, tag="sg4")
                for sub in range(2):
                    lc = pair * 2 + sub
                    nc.vector.tensor_copy(
                        out=sgsb[:, sub, :],
                        in_=s_all[:, lc, mi:mi + 1].to_broadcast((P, L)))
                nc.sync.dma_start(
                    out=h['sg4'][mi, pair * 256:(pair + 1) * 256, :].rearrange(
                        "(c p) s -> p c s", p=P),
                    in_=sgsb)


_PROGRAM = None


def _build_program():
    global _PROGRAM
    if _PROGRAM is not None:
        return _PROGRAM
    import concourse.mybir as mybir
    import concourse.tile as tile
    from concourse import bacc
    from concourse.masks import make_identity

    f32 = mybir.dt.float32
    f32r = mybir.dt.float32r
    bf16 = mybir.dt.bfloat16

    nc = bacc.Bacc(
        "TRN2", target_bir_lowering=False, debug=False, num_devices=8)
    h = {}
    h['q'] = nc.declare_dram_parameter("q", [L, H * E], f32r, isOutput=False)
    h['k'] = nc.declare_dram_parameter("k", [L, H * E], f32r, isOutput=False)
    h['v'] = nc.declare_dram_parameter("v", [L, 4, 65], bf16, isOutput=False)
    h['sig'] = nc.declare_dram_parameter("sig", [L, 4], f32, isOutput=False)
    h['wls'] = nc.declare_dram_parameter("wls", [L, 16], f32r, isOutput=False)
    h['mre'] = nc.declare_dram_parameter("mre", [H * E, FHE], f32r, isOutput=False)
    h['mim'] = nc.declare_dram_parameter("mim", [H * E, FHE], f32r, isOutput=False)
    h['rstk'] = nc.declare_dram_parameter("rstk", [4, 256, L], f32r, isOutput=False)
    h['gp'] = nc.declare_dram_parameter("gp", [P, L], f32r, isOutput=False)
    h['xarrd'] = nc.dram_tensor("xarrd", [2, P, UWD], f32r)
    h['rzd'] = nc.dram_tensor("rzd", [1, L], f32)
    h['vout'] = nc.declare_dram_parameter("vout", [L, 4, D], f32, isOutput=True)
    h['ser'] = nc.declare_dram_parameter("ser", [4, L, L], f32, isOutput=True)
    h['pri'] = nc.declare_dram_parameter("pri", [4, L, L], f32, isOutput=True)
    h['sg4'] = nc.declare_dram_parameter("sg4", [4, L, L], f32, isOutput=True)

    with tile.TileContext(nc) as tc:
        _emit(nc, tc, mybir, make_identity, h)
    nc.compile()
    _PROGRAM = nc
    return nc


def kernel(queries, keys, values, sigma):
    import ml_dtypes
    from concourse.bass_utils import run_bass_kernel_spmd

    queries = np.asarray(queries, np.float32)
    keys = np.asarray(keys, np.float32)
    values = np.asarray(values, np.float32)
    sigma = np.asarray(sigma, np.float32)

    nc = _build_program()

    wls = _build_wls()
    mre, mim = _build_m()
    gp = _build_gp()
    rstk_half = {0: _build_rstk([0, 1, 2, 3]), 1: _build_rstk([4, 5, 6, 7])}

    in_maps = []
    for c in range(8):
        b, mh = c // 2, c % 2
        m0 = 4 * mh
        vsh = np.empty((L, 4, 65), np.float32)
        vsh[:, :, :64] = values[b][:, m0:m0 + 4, :]
        vsh[:, :, 64] = 1.0
        in_maps.append(dict(
            q=np.ascontiguousarray(queries[b].reshape(L, H * E)),
            k=np.ascontiguousarray(keys[b].reshape(L, H * E)),
            v=vsh.astype(ml_dtypes.bfloat16),
            sig=np.ascontiguousarray(sigma[b][:, m0:m0 + 4]),
            wls=wls, mre=mre, mim=mim, rstk=rstk_half[mh], gp=gp,
        ))

    res = run_bass_kernel_spmd(nc, in_maps, core_ids=list(range(8))).results

    V = np.empty((B, L, H, D), np.float32)
    series = np.empty((B, H, L, L), np.float32)
    prior = np.empty((B, H, L, L), np.float32)
    sigma4 = np.empty((B, H, L, L), np.float32)
    for c in range(8):
        b, m0 = c // 2, 4 * (c % 2)
        V[b][:, m0:m0 + 4, :] = res[c]['vout']
        series[b, m0:m0 + 4] = res[c]['ser'].transpose(0, 2, 1)
        prior[b, m0:m0 + 4] = res[c]['pri']
        sigma4[b, m0:m0 + 4] = res[c]['sg4']
    return V, series, prior, sigma4
